# revision 14
# baseline (speedup 1.0000x reference)
"""Trainium2 Bass kernel for nn_CNSYN_59528246723247.

Data-parallel over batch across 8 NeuronCores (64 batches/core).
Each core:
  - gathers entity / context embeddings with indirect DMA
  - context aggregation: scores on DVE, normalization + weighted sum via PE
    block matmuls over 12-row groups laid out (row,ctx)-flat on 120 partitions
  - Q1 MLPs feature-major on PE; masked sum over S via transposed-L2 +
    mask-matrix matmuls
  - Q2 MLPs feature-major on PE with streamed weights
Outputs gathered on host into the reference's 4-tuple.
"""

import sys

sys.path.insert(0, "/opt/trn_rl_repo")

from contextlib import ExitStack

import numpy as np

import concourse.bass as bass
import concourse.mybir as mybir
import concourse.tile as tile
from concourse import bacc
from concourse.bass import IndirectOffsetOnAxis
from concourse.bass_utils import run_bass_kernel_spmd

# ---------------------------------------------------------------- dimensions
B, S, C, E = 512, 64, 10, 100
V, NH, CH = 100000, 1024, 2048
CH2 = CH // 2
NCORES = 8
BC = B // NCORES            # 64 batches per core
R_REAL = BC * S + BC        # 4160 rows per core: 4096 set + 64 inst
R2 = 4224                   # 33*128, padded row count
NT = R2 // 128              # 33 row chunks everywhere

f32 = mybir.dt.float32
i32 = mybir.dt.int32
AF = mybir.ActivationFunctionType
ALU = mybir.AluOpType
AX = mybir.AxisListType

_CACHE = {}


# ---------------------------------------------------------------- program
def build_program():
    if "nc" in _CACHE:
        return _CACHE["nc"]

    nc = bacc.Bacc("TRN2", debug=False, target_bir_lowering=False)

    # ---- DRAM parameters
    emb = nc.dram_tensor("emb", [V, E], f32, kind="ExternalInput")
    ca32 = nc.dram_tensor("ca32", [V, C], i32, kind="ExternalInput")
    ids_ch = nc.dram_tensor("ids_ch", [128, NT], i32, kind="ExternalInput")

    w1_d = nc.dram_tensor("w1", [E, E], f32, kind="ExternalInput")
    w2b_d = nc.dram_tensor("w2b", [E + 1, NH], f32, kind="ExternalInput")
    w1h_d = nc.dram_tensor("w1h", [E, E], f32, kind="ExternalInput")
    w2bh_d = nc.dram_tensor("w2bh", [E + 1, NH], f32, kind="ExternalInput")

    q2w1_d = nc.dram_tensor("q2w1", [NH, CH], f32, kind="ExternalInput")
    q2w2_d = nc.dram_tensor("q2w2", [CH, CH2], f32, kind="ExternalInput")
    q2hw1_d = nc.dram_tensor("q2hw1", [NH, CH], f32, kind="ExternalInput")
    q2hw2_d = nc.dram_tensor("q2hw2", [CH, CH2], f32, kind="ExternalInput")

    b1c_d = nc.dram_tensor("b1c", [128, 16], f32, kind="ExternalInput")
    b2c_d = nc.dram_tensor("b2c", [128, 8], f32, kind="ExternalInput")
    w3c_d = nc.dram_tensor("w3c", [128, 8], f32, kind="ExternalInput")
    b3_d = nc.dram_tensor("b3", [1, 1], f32, kind="ExternalInput")
    b1ch_d = nc.dram_tensor("b1ch", [128, 16], f32, kind="ExternalInput")
    b2ch_d = nc.dram_tensor("b2ch", [128, 8], f32, kind="ExternalInput")
    w3ch_d = nc.dram_tensor("w3ch", [128, 8], f32, kind="ExternalInput")
    b3h_d = nc.dram_tensor("b3h", [1, 1], f32, kind="ExternalInput")

    gmask_d = nc.dram_tensor("gmask", [128, 64], f32, kind="ExternalInput")
    e64_d = nc.dram_tensor("e64", [128, 64], f32, kind="ExternalInput")
    i128_d = nc.dram_tensor("i128", [128, 128], f32, kind="ExternalInput")

    out_d = nc.dram_tensor("out", [2, 2 * BC], f32, kind="ExternalOutput")

    with tile.TileContext(nc) as tc, ExitStack() as ctx:
        const = ctx.enter_context(tc.tile_pool(name="const", bufs=1))
        gat = ctx.enter_context(tc.tile_pool(name="gat", bufs=2))
        work = ctx.enter_context(tc.tile_pool(name="work", bufs=2))
        big = ctx.enter_context(tc.tile_pool(name="big", bufs=1))
        y2p = ctx.enter_context(tc.tile_pool(name="y2p", bufs=2))
        qwp = ctx.enter_context(tc.tile_pool(name="qwp", bufs=2))
        q2wk = ctx.enter_context(tc.tile_pool(name="q2wk", bufs=1))

        # ---- load constants / small weights to SBUF
        ids_sb = const.tile([128, NT], i32)
        nc.sync.dma_start(ids_sb[:], ids_ch[:])
        w1_sb = const.tile([E, E], f32)
        nc.sync.dma_start(w1_sb[:], w1_d[:])
        w2b_sb = const.tile([E + 1, NH], f32)
        nc.sync.dma_start(w2b_sb[:], w2b_d[:])
        w1h_sb = const.tile([E, E], f32)
        nc.sync.dma_start(w1h_sb[:], w1h_d[:])
        w2bh_sb = const.tile([E + 1, NH], f32)
        nc.sync.dma_start(w2bh_sb[:], w2bh_d[:])
        gmask_sb = const.tile([128, 64], f32)
        nc.sync.dma_start(gmask_sb[:], gmask_d[:])
        e64_sb = const.tile([128, 64], f32)
        nc.sync.dma_start(e64_sb[:], e64_d[:])
        i128_sb = const.tile([128, 128], f32)
        nc.sync.dma_start(i128_sb[:], i128_d[:])
        b1c_sb = const.tile([128, 16], f32)
        nc.sync.dma_start(b1c_sb[:], b1c_d[:])
        b2c_sb = const.tile([128, 8], f32)
        nc.sync.dma_start(b2c_sb[:], b2c_d[:])
        w3c_sb = const.tile([128, 8], f32)
        nc.sync.dma_start(w3c_sb[:], w3c_d[:])
        b3_sb = const.tile([1, 1], f32)
        nc.sync.dma_start(b3_sb[:], b3_d[:])
        b1ch_sb = const.tile([128, 16], f32)
        nc.sync.dma_start(b1ch_sb[:], b1ch_d[:])
        b2ch_sb = const.tile([128, 8], f32)
        nc.sync.dma_start(b2ch_sb[:], b2ch_d[:])
        w3ch_sb = const.tile([128, 8], f32)
        nc.sync.dma_start(w3ch_sb[:], w3ch_d[:])
        b3h_sb = const.tile([1, 1], f32)
        nc.sync.dma_start(b3h_sb[:], b3h_d[:])

        # xT activations, feature-major [E, R2]; columns = rows
        xt_set = big.tile([E, R2], f32)
        xt_ctx = big.tile([E, R2], f32)

        # ---------------- phase A: gathers + context aggregation + xT build
        # HW indirect DMA uses ONE dynamic index per partition; everything is
        # organized in 33 chunks of 128 rows.
        with tc.tile_pool(name="psX", bufs=2, space="PSUM") as psX:
            for t in range(NT):
                c0 = t * 128
                ctxids = gat.tile([128, C], i32)
                nc.gpsimd.indirect_dma_start(
                    out=ctxids[:], out_offset=None, in_=ca32[:, :],
                    in_offset=IndirectOffsetOnAxis(
                        ap=ids_sb[:, t:t + 1], axis=0),
                )
                ent = gat.tile([128, E], f32)
                nc.gpsimd.indirect_dma_start(
                    out=ent[:], out_offset=None, in_=emb[:, :],
                    in_offset=IndirectOffsetOnAxis(
                        ap=ids_sb[:, t:t + 1], axis=0),
                )
                ctx_t = gat.tile([128, C * E], f32)
                for c in range(C):
                    nc.gpsimd.indirect_dma_start(
                        out=ctx_t[:, c * E:(c + 1) * E],
                        out_offset=None, in_=emb[:, :],
                        in_offset=IndirectOffsetOnAxis(
                            ap=ctxids[:, c:c + 1], axis=0),
                    )

                # scores s[p,c] = <ctx[p,c,:], ent[p,:]> on DVE
                prod = work.tile([128, C * E], f32)
                nc.vector.tensor_tensor(
                    prod[:].rearrange("p (c d) -> p c d", c=C),
                    ctx_t[:].rearrange("p (c d) -> p c d", c=C),
                    ent[:].unsqueeze(1).to_broadcast([128, C, E]),
                    op=ALU.mult,
                )
                s_all = work.tile([128, C], f32)
                nc.vector.tensor_reduce(
                    s_all[:], prod[:].rearrange("p (c d) -> p c d", c=C),
                    axis=AX.X, op=ALU.add,
                )
                z = work.tile([128, 1], f32)
                nc.vector.tensor_reduce(z[:], s_all[:], axis=AX.X, op=ALU.add)
                rz = work.tile([128, 1], f32)
                nc.vector.reciprocal(rz[:], z[:])
                alpha = work.tile([128, C], f32)
                nc.vector.tensor_scalar(alpha[:], s_all[:], rz[:], None,
                                        op0=ALU.mult)

                # scaled[p, c, :] = alpha[p, c] * ctx[p, c, :]
                scaled = work.tile([128, C * E], f32)
                nc.vector.tensor_tensor(
                    scaled[:].rearrange("p (c d) -> p c d", c=C),
                    ctx_t[:].rearrange("p (c d) -> p c d", c=C),
                    alpha[:].unsqueeze(2).to_broadcast([128, C, E]),
                    op=ALU.mult,
                )
                # xT_ctx chunk = sum_c scaled_c^T  (PE transpose-accumulate)
                xtc_ps = psX.tile([E, 128], f32)
                for c in range(C):
                    nc.tensor.matmul(
                        xtc_ps[:], lhsT=scaled[:, c * E:(c + 1) * E],
                        rhs=i128_sb[:],
                        start=(c == 0), stop=(c == C - 1),
                    )
                xts_ps = psX.tile([E, 128], f32)
                nc.tensor.matmul(xts_ps[:], lhsT=ent[:], rhs=i128_sb[:],
                                 start=True, stop=True)
                nc.scalar.copy(xt_ctx[:, c0:c0 + 128], xtc_ps[:])
                nc.scalar.copy(xt_set[:, c0:c0 + 128], xts_ps[:])

        # ---------------- phases B/C: Q1 MLPs + masked segment sums
        def q1_path(xt_sb, w1s, w2bs, name):
            with (
                tc.tile_pool(name=name + "ps1", bufs=2, space="PSUM") as ps1,
                tc.tile_pool(name=name + "ps2", bufs=2, space="PSUM") as ps2,
                tc.tile_pool(name=name + "acc", bufs=1, space="PSUM") as pacc,
            ):
                h1 = big.tile([E + 1, R2], f32, name=name + "_h1")
                # engine ops need 32-aligned start partition: set rows 96..100
                # to 1.0; the L1 relu overwrites rows 96..99 afterwards.
                nc.vector.memset(h1[96:E + 1, :], 1.0)
                for j in range(0, R2, 512):
                    w = min(512, R2 - j)
                    ps = ps1.tile([E, 512], f32, name=name + "_l1")
                    nc.tensor.matmul(ps[:, :w], lhsT=w1s[:],
                                     rhs=xt_sb[:, j:j + w],
                                     start=True, stop=True)
                    nc.scalar.activation(h1[0:E, j:j + w], ps[:, :w], AF.Relu)

                segacc = pacc.tile([128, 512], f32, name=name + "_seg")
                instacc = pacc.tile([128, 512], f32, name=name + "_ins")
                for t in range(NT):
                    ya = ps2.tile([128, 512], f32, name=name + "_l2a")
                    yb = ps2.tile([128, 512], f32, name=name + "_l2b")
                    lhsT = h1[:, 128 * t:128 * (t + 1)]
                    nc.tensor.matmul(ya[:], lhsT=lhsT, rhs=w2bs[:, 0:512],
                                     start=True, stop=True)
                    nc.tensor.matmul(yb[:], lhsT=lhsT, rhs=w2bs[:, 512:NH],
                                     start=True, stop=True)
                    y2 = y2p.tile([128, NH], f32, name=name + "_y2")
                    nc.scalar.activation(y2[:, 0:512], ya[:], AF.Relu)
                    nc.scalar.activation(y2[:, 512:NH], yb[:], AF.Relu)
                    if t < 32:
                        for f in range(8):
                            nc.tensor.matmul(
                                segacc[:, 64 * f + 2 * t:64 * f + 2 * t + 2],
                                lhsT=y2[:, 128 * f:128 * (f + 1)],
                                rhs=gmask_sb[:, 2 * t:2 * t + 2],
                                start=True, stop=True,
                            )
                    else:
                        for f in range(8):
                            nc.tensor.matmul(
                                instacc[:, 64 * f:64 * (f + 1)],
                                lhsT=y2[:, 128 * f:128 * (f + 1)],
                                rhs=e64_sb[:],
                                start=True, stop=True,
                            )

                # q2 inputs: [embed | embed + inst-embed], feature-major blocks
                iT = q2wk.tile([128, 512], f32, name=name + "_iT")
                nc.scalar.copy(iT[:], instacc[:])
                x2 = q2wk.tile([128, NH], f32, name=name + "_x2")
                for f in range(8):
                    nc.scalar.copy(x2[:, 128 * f:128 * f + 64],
                                   segacc[:, 64 * f:64 * (f + 1)])
                    nc.vector.tensor_tensor(
                        x2[:, 128 * f + 64:128 * (f + 1)],
                        segacc[:, 64 * f:64 * (f + 1)],
                        iT[:, 64 * f:64 * (f + 1)],
                        op=ALU.add,
                    )
                return x2

        x2_set = q1_path(xt_set, w1_sb, w2b_sb, "qs")
        x2_ctx = q1_path(xt_ctx, w1h_sb, w2bh_sb, "qc")

        # ---------------- phase D: Q2 MLPs
        def q2_mlp(x2, q2w1, q2w2, b1s, b2s, w3s, b3s, out_row, name):
            with (
                tc.tile_pool(name=name + "ps", bufs=2, space="PSUM") as psq,
                tc.tile_pool(name=name + "ps3", bufs=1, space="PSUM") as psq3,
            ):
                hq = q2wk.tile([128, CH], f32, name=name + "_hq")
                for m in range(16):
                    wt = qwp.tile([128, NH], f32, name="q2w1t")
                    nc.sync.dma_start(
                        wt[:].rearrange("p (k c) -> p k c", k=8),
                        q2w1[:, 128 * m:128 * (m + 1)].rearrange(
                            "(k p) c -> p k c", p=128
                        ),
                    )
                    ps = psq.tile([128, 128], f32, name=name + "_p1")
                    for k in range(8):
                        nc.tensor.matmul(
                            ps[:],
                            lhsT=wt[:, 128 * k:128 * (k + 1)],
                            rhs=x2[:, 128 * k:128 * (k + 1)],
                            start=(k == 0), stop=(k == 7),
                        )
                    nc.scalar.activation(hq[:, 128 * m:128 * (m + 1)], ps[:],
                                         AF.Relu, bias=b1s[:, m:m + 1])
                h2 = q2wk.tile([128, CH2], f32, name=name + "_h2")
                for m in range(8):
                    wt2 = qwp.tile([128, CH], f32, name="q2w2t")
                    nc.sync.dma_start(
                        wt2[:].rearrange("p (k c) -> p k c", k=16),
                        q2w2[:, 128 * m:128 * (m + 1)].rearrange(
                            "(k p) c -> p k c", p=128
                        ),
                    )
                    ps = psq.tile([128, 128], f32, name=name + "_p2")
                    for k in range(16):
                        nc.tensor.matmul(
                            ps[:],
                            lhsT=wt2[:, 128 * k:128 * (k + 1)],
                            rhs=hq[:, 128 * k:128 * (k + 1)],
                            start=(k == 0), stop=(k == 15),
                        )
                    nc.scalar.activation(h2[:, 128 * m:128 * (m + 1)], ps[:],
                                         AF.Relu, bias=b2s[:, m:m + 1])
                ps3 = psq3.tile([1, 128], f32, name=name + "_p3")
                for k in range(8):
                    nc.tensor.matmul(
                        ps3[:],
                        lhsT=w3s[:, k:k + 1],
                        rhs=h2[:, 128 * k:128 * (k + 1)],
                        start=(k == 0), stop=(k == 7),
                    )
                osb = q2wk.tile([1, 128], f32, name=name + "_o")
                nc.scalar.activation(osb[:], ps3[:], AF.Identity, bias=b3s[:])
                nc.sync.dma_start(out_row, osb[:])

        q2_mlp(x2_set, q2w1_d, q2w2_d, b1c_sb, b2c_sb, w3c_sb, b3_sb,
               out_d[0:1, :], "q2s")
        q2_mlp(x2_ctx, q2hw1_d, q2hw2_d, b1ch_sb, b2ch_sb, w3ch_sb, b3h_sb,
               out_d[1:2, :], "q2h")

    nc.compile()
    _CACHE["nc"] = nc
    return nc


# ---------------------------------------------------------------- host prep
def _shared_consts():
    if "consts" in _CACHE:
        return _CACHE["consts"]
    c = {
        "e64": np.eye(128, 64, dtype=np.float32),
        "i128": np.eye(128, dtype=np.float32),
    }
    _CACHE["consts"] = c
    return c


def make_in_maps(inputs):
    """inputs: dict of FULL numpy arrays keyed as in setup_inputs()."""
    inp = {k: np.asarray(v) for k, v in inputs.items()}
    set_ids = inp["set_ids"].astype(np.int32)
    inst_ids = inp["inst_ids"].astype(np.int32)
    ca32 = np.ascontiguousarray(inp["contex_array"].astype(np.int32))
    emb = np.ascontiguousarray(inp["emb"].astype(np.float32))

    shared = {
        "emb": emb,
        "ca32": ca32,
        "w1": np.ascontiguousarray(inp["q1_w1"].astype(np.float32)),
        "w2b": np.ascontiguousarray(
            np.vstack([inp["q1_w2"], inp["q1_b2"][None, :]]).astype(np.float32)
        ),
        "w1h": np.ascontiguousarray(inp["q1h_w1"].astype(np.float32)),
        "w2bh": np.ascontiguousarray(
            np.vstack([inp["q1h_w2"], inp["q1h_b2"][None, :]]).astype(np.float32)
        ),
        "q2w1": np.ascontiguousarray(inp["q2_w1"].astype(np.float32)),
        "q2w2": np.ascontiguousarray(inp["q2_w2"].astype(np.float32)),
        "q2hw1": np.ascontiguousarray(inp["q2h_w1"].astype(np.float32)),
        "q2hw2": np.ascontiguousarray(inp["q2h_w2"].astype(np.float32)),
        "b1c": np.ascontiguousarray(
            inp["q2_b1"].astype(np.float32).reshape(16, 128).T),
        "b2c": np.ascontiguousarray(
            inp["q2_b2"].astype(np.float32).reshape(8, 128).T),
        "w3c": np.ascontiguousarray(
            inp["q2_w3"].astype(np.float32).reshape(8, 128).T),
        "b3": inp["q2_b3"].astype(np.float32).reshape(1, 1),
        "b1ch": np.ascontiguousarray(
            inp["q2h_b1"].astype(np.float32).reshape(16, 128).T),
        "b2ch": np.ascontiguousarray(
            inp["q2h_b2"].astype(np.float32).reshape(8, 128).T),
        "w3ch": np.ascontiguousarray(
            inp["q2h_w3"].astype(np.float32).reshape(8, 128).T),
        "b3h": inp["q2h_b3"].astype(np.float32).reshape(1, 1),
    }
    shared.update(_shared_consts())

    in_maps = []
    for c in range(NCORES):
        sid = set_ids[c * BC:(c + 1) * BC]          # [64, 64]
        iid = inst_ids[c * BC:(c + 1) * BC, 0]      # [64]
        ids_flat = np.concatenate(
            [sid.reshape(-1), iid,
             np.ones(R2 - R_REAL, np.int32)]).astype(np.int32)
        ids_ch = np.ascontiguousarray(ids_flat.reshape(NT, 128).T)
        mask = (sid != 0).astype(np.float32)        # [64, 64]
        gmask = np.zeros((128, 64), np.float32)
        for t in range(32):
            gmask[0:64, 2 * t] = mask[2 * t, :]
            gmask[64:128, 2 * t + 1] = mask[2 * t + 1, :]
        m = dict(shared)
        m["ids_ch"] = ids_ch
        m["gmask"] = gmask
        in_maps.append(m)
    return in_maps


def assemble_outputs(results):
    """results: list (per core) of dicts with 'out' [2, 128]."""
    setQ2 = np.zeros((B, 1), np.float32)
    setInst = np.zeros((B, 1), np.float32)
    ctxHat = np.zeros((B, 1), np.float32)
    ctxInstHat = np.zeros((B, 1), np.float32)
    for c in range(NCORES):
        o = np.asarray(results[c]["out"])
        setQ2[c * BC:(c + 1) * BC, 0] = o[0, 0:BC]
        setInst[c * BC:(c + 1) * BC, 0] = o[0, BC:2 * BC]
        ctxHat[c * BC:(c + 1) * BC, 0] = o[1, 0:BC]
        ctxInstHat[c * BC:(c + 1) * BC, 0] = o[1, BC:2 * BC]
    return (setQ2, setInst, ctxHat, ctxInstHat)


def run_cores(inputs, trace=False, **kw):
    nc = build_program()
    in_maps = make_in_maps(inputs)
    res = run_bass_kernel_spmd(nc, in_maps, list(range(NCORES)),
                               trace=trace, **kw)
    return assemble_outputs(res.results), res


def kernel(**inputs):
    outs, _ = run_cores(inputs, trace=False)
    return outs
